# revision 1
# baseline (speedup 1.0000x reference)
"""Trainium2 Bass kernel for an RNN-T joint network.

Computation (per batch element b):
    enc_p  = enc_out @ W_enc + b_enc                      # (T, J)
    pred_p = pred_out @ W_pred + b_pred                   # (U, J)
    joint  = tanh(enc_p[:, None, :] + pred_p[None, :, :]) # (T, U, J)
    logits = joint @ W_joint + b_joint                    # (T, U, V)

Distribution: data-parallel over the batch dim B=8, one batch element per
NeuronCore.  Inside one core everything is kept J-major (J on the SBUF
partition dim) so that:
  * the broadcast add + tanh fuses into ONE ScalarE activation per
    (j-tile, u): tanh(enc_pT[j, :] + bias) with bias = pred_pT[j, u]
    (a per-partition scalar, which the ACT instruction supports natively),
  * the vocab matmul contraction (over J) has J on the partition dim as
    the tensor engine requires.
Stage-2 (the 17-GFLOP vocab projection) runs in bf16 (fp32 accumulate in
PSUM); stage-1 runs in fp32.  b_joint is added by VectorE while draining
PSUM -> SBUF; output rows are stored u-major so each DMA writes 4KB
contiguous DRAM chunks.
"""

from contextlib import ExitStack

import numpy as np

import concourse.bacc as bacc
import concourse.mybir as mybir
import concourse.tile as tile
import concourse.bass_utils as bass_utils
from concourse.masks import make_identity

N_CORES = 8
T, U, J, V = 256, 64, 512, 1024
DE, DP = 512, 640
KJ = J // 128    # j-tiles
KE = DE // 128   # enc contraction tiles
KP = DP // 128   # pred contraction tiles
HT = T // 128    # t-halves
NV = V // 512    # vocab halves (one PSUM bank each)
F32 = mybir.dt.float32
F32R = mybir.dt.float32r
BF16 = mybir.dt.bfloat16

_CACHE: dict = {}
# Ablation switches for cost-model experiments (bench/devloop only).
_OPTS: dict = {"no_store": False, "no_mm2": False, "no_act": False, "no_drain": False,
               "s1_bf16": False, "prewarm": 0,
               "ps_bufs": 8, "joint_bufs": 12, "out_bufs": 3,
               # f32r stage-2 gives L2 rel err 1.4e-4 (vs 2.3e-3 bf16) but
               # measures ~312us/iter vs ~233us: fp32 weight loads disable
               # FWL and serialize LDWEIGHTS. bf16 is the ridge-optimal pick.
               "mm2_f32r": False}


def _emit(tc, nc, d, repeats=1):
    act = mybir.ActivationFunctionType
    with ExitStack() as ctx:
        const = ctx.enter_context(tc.tile_pool(name="const", bufs=1))
        stg = ctx.enter_context(tc.tile_pool(name="stg", bufs=1 if repeats == 1 else 2))
        ps = ctx.enter_context(tc.tile_pool(name="ps", bufs=_OPTS["ps_bufs"], space="PSUM"))
        joint_pool = ctx.enter_context(tc.tile_pool(name="jp", bufs=_OPTS["joint_bufs"]))
        out_pool = ctx.enter_context(tc.tile_pool(name="op", bufs=_OPTS["out_bufs"]))
        s1dt = BF16 if _OPTS["s1_bf16"] else F32

        # ---------------- loads ----------------
        enc_sb = const.tile([128, HT, DE], F32, tag="enc_sb")
        nc.gpsimd.dma_start(enc_sb[:], d["enc_out"].ap().rearrange("(h p) d -> p h d", p=128))
        pred_sb = const.tile([U, DP], F32, tag="pred_sb")
        nc.gpsimd.dma_start(pred_sb[:], d["pred_out"].ap())
        wenc_sb = const.tile([128, KE, J], F32, tag="wenc_sb")
        nc.gpsimd.dma_start(wenc_sb[:], d["W_enc"].ap().rearrange("(k p) j -> p k j", p=128))
        wpred_sb = const.tile([128, KP, J], F32, tag="wpred_sb")
        nc.gpsimd.dma_start(wpred_sb[:], d["W_pred"].ap().rearrange("(k p) j -> p k j", p=128))
        mmdt = F32R if _OPTS["mm2_f32r"] else BF16
        if _OPTS["mm2_f32r"]:
            # fp32r streams at bf16 rate for N>=256 with ~fp32 accuracy;
            # W_joint loads straight from DRAM, no cast needed.
            wj_sb = const.tile([128, KJ, V], F32R, tag="wj_sb")
            nc.gpsimd.dma_start(wj_sb[:], d["W_joint"].ap().rearrange("(k p) v -> p k v", p=128))
        else:
            wj_f32 = const.tile([128, KJ, V], F32, tag="wj_f32")
            nc.gpsimd.dma_start(wj_f32[:], d["W_joint"].ap().rearrange("(k p) v -> p k v", p=128))
            wj_sb = const.tile([128, KJ, V], BF16, tag="wj_sb")
            nc.vector.tensor_copy(wj_sb[:], wj_f32[:])

        benc_sb = const.tile([128, KJ], F32, tag="benc_sb")
        nc.gpsimd.dma_start(benc_sb[:], d["b_enc"].ap().rearrange("(k p) -> p k", p=128))
        bpred_sb = const.tile([128, KJ], F32, tag="bpred_sb")
        nc.gpsimd.dma_start(bpred_sb[:], d["b_pred"].ap().rearrange("(k p) -> p k", p=128))
        bsum = const.tile([128, KJ], F32, tag="bsum")
        nc.vector.tensor_add(bsum[:], benc_sb[:], bpred_sb[:])
        # b_joint replicated into every partition (DMA reads it 128x).
        bj_sb = const.tile([128, V], F32, tag="bj_sb")
        nc.sync.dma_start(bj_sb[:], d["b_joint"].ap().unsqueeze(0).to_broadcast((128, V)))

        ident = const.tile([128, 128], s1dt, tag="ident")
        make_identity(nc, ident[:])

        if _OPTS["s1_bf16"]:
            enc_c = const.tile([128, HT, DE], BF16, tag="enc_c")
            nc.vector.tensor_copy(enc_c[:], enc_sb[:])
            enc_sb = enc_c
            pred_c = const.tile([U, DP], BF16, tag="pred_c")
            nc.vector.tensor_copy(pred_c[:], pred_sb[:])
            pred_sb = pred_c
            wenc_c = const.tile([128, KE, J], BF16, tag="wenc_c")
            nc.vector.tensor_copy(wenc_c[:], wenc_sb[:])
            wenc_sb = wenc_c
            wpred_c = const.tile([128, KP, J], BF16, tag="wpred_c")
            nc.vector.tensor_copy(wpred_c[:], wpred_sb[:])
            wpred_sb = wpred_c

        if _OPTS["prewarm"]:
            # keep PE busy during the input-DMA phase so HAM un-throttles
            # before the real matmul stream begins
            pw = ps.tile([128, 128], s1dt, tag="ps", name="prewarm")
            for i in range(_OPTS["prewarm"]):
                nc.tensor.transpose(pw[:], ident[:], ident[:])

        out_ap = d["logits"].ap()
        for rep in range(repeats):
            # ---------------- transposes (PE) ----------------
            # enc_t[dp, dk, t] = enc_out[t, dk*128+dp]
            enc_t = stg.tile([128, KE, T], s1dt, tag="enc_t", name=f"enc_t_{rep}")
            for k in range(KE):
                pt = ps.tile([128, T], s1dt, tag="ps", name=f"tr_enc_{rep}_{k}")
                for h in range(HT):
                    nc.tensor.transpose(
                        pt[:, h * 128:(h + 1) * 128],
                        enc_sb[:, h, k * 128:(k + 1) * 128],
                        ident[:],
                    )
                nc.scalar.copy(enc_t[:, k, :], pt[:])
            # pred_t[dp, dk, u] = pred_out[u, dk*128+dp]
            pred_t = stg.tile([128, KP, U], s1dt, tag="pred_t", name=f"pred_t_{rep}")
            for k in range(KP):
                pt = ps.tile([128, U], s1dt, tag="ps", name=f"tr_pred_{rep}_{k}")
                nc.tensor.transpose(pt[:], pred_sb[:, k * 128:(k + 1) * 128], ident[:U, :U])
                nc.scalar.copy(pred_t[:, k, :], pt[:])

            # ---------------- stage-1 projections (fp32) ----------------
            # enc_p[jp, jt, t] = (enc_out @ W_enc)[t, jt*128+jp]
            enc_p = stg.tile([128, KJ, T], F32, tag="enc_p", name=f"enc_p_{rep}")
            for j in range(KJ):
                pt = ps.tile([128, T], F32, tag="ps", name=f"mm_enc_{rep}_{j}")
                for k in range(KE):
                    nc.tensor.matmul(
                        pt[:],
                        wenc_sb[:, k, j * 128:(j + 1) * 128],
                        enc_t[:, k, :],
                        start=(k == 0),
                        stop=(k == KE - 1),
                    )
                nc.scalar.copy(enc_p[:, j, :], pt[:])
            # pred_p additionally carries b_enc + b_pred (per-partition scalar).
            pred_p = stg.tile([128, KJ, U], F32, tag="pred_p", name=f"pred_p_{rep}")
            for j in range(KJ):
                pt = ps.tile([128, U], F32, tag="ps", name=f"mm_pred_{rep}_{j}")
                for k in range(KP):
                    nc.tensor.matmul(
                        pt[:],
                        wpred_sb[:, k, j * 128:(j + 1) * 128],
                        pred_t[:, k, :],
                        start=(k == 0),
                        stop=(k == KP - 1),
                    )
                nc.vector.tensor_scalar_add(pred_p[:, j, :], pt[:], bsum[:, j:j + 1])

            # ---------------- main loop over u ----------------
            for u in range(U):
                jt = []
                for j in range(KJ):
                    jtile = joint_pool.tile([128, T], mmdt, tag="joint", name=f"joint_{rep}_{u}_{j}")
                    if not _OPTS["no_act"]:
                        nc.scalar.activation(
                            jtile[:], enc_p[:, j, :], act.Tanh,
                            bias=pred_p[:, j, u:u + 1], scale=1.0,
                        )
                    else:
                        nc.gpsimd.memset(jtile[:, :1], 0.0)
                    jt.append(jtile)
                ot = out_pool.tile([128, HT, V], F32, tag="out", name=f"out_{rep}_{u}")
                for h in range(HT):
                    for vh in range(NV):
                        pt = ps.tile([128, 512], F32, tag="ps", name=f"mm_{rep}_{u}_{h}_{vh}")
                        if not _OPTS["no_mm2"]:
                            for j in range(KJ):
                                nc.tensor.matmul(
                                    pt[:],
                                    jt[j][:, h * 128:(h + 1) * 128],
                                    wj_sb[:, j, vh * 512:(vh + 1) * 512],
                                    start=(j == 0),
                                    stop=(j == KJ - 1),
                                )
                        else:
                            nc.tensor.matmul(
                                pt[:], jt[0][:, h * 128:(h + 1) * 128],
                                wj_sb[:, 0, vh * 512:(vh + 1) * 512],
                                start=True, stop=True,
                            )
                        if not _OPTS["no_drain"]:
                            nc.vector.tensor_tensor(
                                ot[:, h, vh * 512:(vh + 1) * 512],
                                pt[:],
                                bj_sb[:, vh * 512:(vh + 1) * 512],
                                mybir.AluOpType.add,
                            )
                        else:
                            nc.vector.tensor_copy(ot[:, h, vh * 512:vh * 512 + 1], pt[:, :1])
                if not _OPTS["no_store"]:
                    nc.sync.dma_start(
                        out_ap[:, u, :].rearrange("(h p) v -> p h v", p=128),
                        ot[:],
                    )


def _build_program(repeats=1):
    nc = bacc.Bacc("TRN2", target_bir_lowering=False, debug=False, num_devices=N_CORES)
    d = {
        "enc_out": nc.dram_tensor("enc_out", (T, DE), F32, kind="ExternalInput"),
        "pred_out": nc.dram_tensor("pred_out", (U, DP), F32, kind="ExternalInput"),
        "W_enc": nc.dram_tensor("W_enc", (DE, J), F32, kind="ExternalInput"),
        "b_enc": nc.dram_tensor("b_enc", (J,), F32, kind="ExternalInput"),
        "W_pred": nc.dram_tensor("W_pred", (DP, J), F32, kind="ExternalInput"),
        "b_pred": nc.dram_tensor("b_pred", (J,), F32, kind="ExternalInput"),
        "W_joint": nc.dram_tensor("W_joint", (J, V),
                                  F32R if _OPTS["mm2_f32r"] else F32,
                                  kind="ExternalInput"),
        "b_joint": nc.dram_tensor("b_joint", (V,), F32, kind="ExternalInput"),
        "logits": nc.dram_tensor("logits", (T, U, V), F32, kind="ExternalOutput"),
    }
    with tile.TileContext(nc) as tc:
        _emit(tc, nc, d, repeats=repeats)
    nc.compile()
    return nc


def kernel(enc_out, pred_out, W_enc, b_enc, W_pred, b_pred, W_joint, b_joint):
    nc = _CACHE.get("nc")
    if nc is None:
        nc = _CACHE["nc"] = _build_program()

    shared = {
        "W_enc": np.ascontiguousarray(W_enc, dtype=np.float32),
        "b_enc": np.ascontiguousarray(b_enc, dtype=np.float32),
        "W_pred": np.ascontiguousarray(W_pred, dtype=np.float32),
        "b_pred": np.ascontiguousarray(b_pred, dtype=np.float32),
        "W_joint": np.ascontiguousarray(W_joint, dtype=np.float32),
        "b_joint": np.ascontiguousarray(b_joint, dtype=np.float32),
    }
    in_maps = [
        {
            "enc_out": np.ascontiguousarray(enc_out[c], dtype=np.float32),
            "pred_out": np.ascontiguousarray(pred_out[c], dtype=np.float32),
            **shared,
        }
        for c in range(N_CORES)
    ]
    res = bass_utils.run_bass_kernel_spmd(nc, in_maps, core_ids=list(range(N_CORES)))
    _CACHE["last_results"] = res
    return np.stack([res.results[c]["logits"] for c in range(N_CORES)])



# revision 18
# speedup vs baseline: 1.1829x; 1.1829x over previous
"""Trainium2 Bass kernel for an RNN-T joint network.

Computation (per batch element b):
    enc_p  = enc_out @ W_enc + b_enc                      # (T, J)
    pred_p = pred_out @ W_pred + b_pred                   # (U, J)
    joint  = tanh(enc_p[:, None, :] + pred_p[None, :, :]) # (T, U, J)
    logits = joint @ W_joint + b_joint                    # (T, U, V)

Distribution: data-parallel over the batch dim B=8, one batch element per
NeuronCore.  J-major layout (J on the SBUF partition dim) so the broadcast
add + tanh fuses into one ScalarE activation per (j-tile, u) and the vocab
matmul contracts over the partition dim.

Key optimizations (steady state measures ~225us/iter vs the ~218us bf16
tensor-engine roofline for the 17.2 GFLOP/core vocab projection):
  * inputs are pre-transposed and pre-cast to bf16 on the host (enc/pred
    arrive as (D, T)/(D, U); weights as bf16) - no PE transposes, no
    on-device casts, half the input DMA bytes.  b_enc+b_pred summed on host.
  * logits stored as bf16 (halves the 64 MiB/core store traffic; host
    upcasts to fp32).  bf16 math keeps L2 rel err ~4e-3, far under the
    2e-2 gate.
  * stores batched 4 u's per DMA -> stores of 2 MiB with 8 KiB contiguous
    DRAM chunks; the last group is split so the unhidden final store is
    small.
  * PSUM as [128, 1024] two-bank tiles; one VectorE drain (bias add,
    fp32->bf16) per (u, t-half).
  * loads split per k/j tile and spread over two DMA queues so stage-1
    starts after ~0.5 MB and the vocab matmul as soon as W_joint[j=0]
    lands; each joint tile is consumed in one 4-matmul burst so ScalarE
    tanh production stays ahead of PE demand.
"""

from contextlib import ExitStack

import numpy as np

import concourse.bacc as bacc
import concourse.mybir as mybir
import concourse.tile as tile
import concourse.bass_utils as bass_utils

N_CORES = 8
T, U, J, V = 256, 64, 512, 1024
DE, DP = 512, 640
KJ = J // 128    # j-tiles
KE = DE // 128   # enc contraction tiles
KP = DP // 128   # pred contraction tiles
HT = T // 128    # t-halves
F32 = mybir.dt.float32
BF16 = mybir.dt.bfloat16

_CACHE: dict = {}
# Ablation switches for bench/devloop only.
_OPTS: dict = {"no_store": False, "no_mm2": False, "no_act": False, "no_drain": False,
               "wide_mm": False,  # N=1024 matmul fails the s3d3 ISA check (PSUM bank)
               "prewarm": 0,
               "group": 4, "ps_bufs": 4, "joint_bufs": 12, "out_bufs": 2}


def _emit(tc, nc, d, repeats=1):
    act = mybir.ActivationFunctionType
    g = _OPTS["group"]
    ng = U // g
    with ExitStack() as ctx:
        const = ctx.enter_context(tc.tile_pool(name="const", bufs=1))
        stg = ctx.enter_context(tc.tile_pool(name="stg", bufs=1 if repeats == 1 else 2))
        joint_pool = ctx.enter_context(tc.tile_pool(name="jp", bufs=_OPTS["joint_bufs"]))
        out_pool = ctx.enter_context(tc.tile_pool(name="op", bufs=_OPTS["out_bufs"]))

        # ---------------- loads (gpsimd queue: stage-1-critical first) ----
        # Split per k-tile so the DMA completions arrive incrementally and
        # stage-1 can start as soon as the first (enc, wenc) slices land.
        enc_t = const.tile([128, KE, T], BF16, tag="enc_t")
        wenc_sb = const.tile([128, KE, J], BF16, tag="wenc_sb")
        enc_ap = d["enc_tT"].ap().rearrange("(k p) t -> p k t", p=128)
        wenc_ap = d["W_enc"].ap().rearrange("(k p) j -> p k j", p=128)
        for k in range(KE):
            nc.gpsimd.dma_start(enc_t[:, k, :], enc_ap[:, k, :])
            nc.gpsimd.dma_start(wenc_sb[:, k, :], wenc_ap[:, k, :])
        # b_joint broadcast (512 KB: DMA reads it 128x) rides the gpsimd
        # queue after the enc slices — needed only by the first drain (~7us).
        bj_sb = const.tile([128, V], F32, tag="bj_sb")
        nc.gpsimd.dma_start(bj_sb[:], d["b_joint"].ap().unsqueeze(0).to_broadcast((128, V)))
        pred_t = const.tile([128, KP, U], BF16, tag="pred_t")
        wpred_sb = const.tile([128, KP, J], BF16, tag="wpred_sb")
        pred_ap = d["pred_tT"].ap().rearrange("(k p) u -> p k u", p=128)
        wpred_ap = d["W_pred"].ap().rearrange("(k p) j -> p k j", p=128)
        for k in range(KP):
            nc.gpsimd.dma_start(pred_t[:, k, :], pred_ap[:, k, :])
            nc.gpsimd.dma_start(wpred_sb[:, k, :], wpred_ap[:, k, :])
        bsum = const.tile([128, KJ], F32, tag="bsum")
        nc.gpsimd.dma_start(bsum[:], d["b_sum"].ap().rearrange("(k p) -> p k", p=128))

        # W_joint alone on the sync queue, split per j-tile: the first mm2
        # can start as soon as wj[j=0] lands (~1.5us).
        wj_sb = const.tile([128, KJ, V], BF16, tag="wj_sb")
        wj_ap = d["W_joint"].ap().rearrange("(k p) v -> p k v", p=128)
        for j in range(KJ):
            nc.sync.dma_start(wj_sb[:, j, :], wj_ap[:, j, :])

        if _OPTS["prewarm"]:
            # Keep PE busy during the input-DMA phase so the HAM clock gate
            # un-throttles (K=8/8) before the real matmul stream begins.
            junk = const.tile([128, 640], BF16, tag="pw_junk")
            nc.gpsimd.memset(junk[:], 0.0)
            with tc.tile_pool(name="pw_ps", bufs=1, space="PSUM") as pwp:
                pw = pwp.tile([128, 512], F32, tag="pw")
                for _ in range(_OPTS["prewarm"]):
                    nc.tensor.matmul(pw[:], junk[:, :128], junk[:, 128:640],
                                     start=True, stop=True)

        out_ap = d["logits"].ap()
        for rep in range(repeats):
            with tc.tile_pool(name=f"ps1_{rep}", bufs=2, space="PSUM") as ps1:
                # ---------------- stage-1 projections ----------------
                # j-outer: enc_p[0] / pred_p[0] complete early so the
                # ScalarE tanh pipeline starts filling immediately.
                # Interleave pred after the first enc j so ACT(u=0) can
                # begin as soon as (enc_p[0], pred_p[0]) exist.
                enc_p = stg.tile([128, KJ, T], F32, tag="enc_p", name=f"enc_p_{rep}")
                pred_p = stg.tile([128, KJ, U], F32, tag="pred_p", name=f"pred_p_{rep}")
                for j in range(KJ):
                    pt = ps1.tile([128, T], F32, tag="ps1mm", name=f"mm_enc_{rep}_{j}")
                    for k in range(KE):
                        nc.tensor.matmul(
                            pt[:],
                            wenc_sb[:, k, j * 128:(j + 1) * 128],
                            enc_t[:, k, :],
                            start=(k == 0),
                            stop=(k == KE - 1),
                        )
                    nc.vector.tensor_copy(enc_p[:, j, :], pt[:])
                    # pred_p[j] right after enc_p[j]; carries b_enc + b_pred.
                    pp = ps1.tile([128, U], F32, tag="ps1pu", name=f"mm_pred_{rep}_{j}")
                    for k in range(KP):
                        nc.tensor.matmul(
                            pp[:],
                            wpred_sb[:, k, j * 128:(j + 1) * 128],
                            pred_t[:, k, :],
                            start=(k == 0),
                            stop=(k == KP - 1),
                        )
                    nc.vector.tensor_scalar_add(pred_p[:, j, :], pp[:], bsum[:, j:j + 1])

            # ---------------- main loop over u, grouped stores ----------
            # Tapered tail: last 4 u's stored in pairs so the final store
            # (the unhidden tail) is small.
            groups = [g] * (ng - 1) + [g - g // 2, g // 2] if g >= 4 else [g] * ng
            assert sum(groups) == U
            with tc.tile_pool(name=f"ps2_{rep}", bufs=_OPTS["ps_bufs"], space="PSUM") as ps2:
                u_base = 0
                for gi, gsz in enumerate(groups):
                    ot = out_pool.tile([128, HT, gsz * V], BF16,
                                       tag=f"out{gsz}", name=f"out_{rep}_{gi}")
                    for ui in range(gsz):
                        u = u_base + ui
                        jt = []
                        for j in range(KJ):
                            jtile = joint_pool.tile([128, T], BF16, tag="joint",
                                                    name=f"joint_{rep}_{u}_{j}")
                            if not _OPTS["no_act"]:
                                nc.scalar.activation(
                                    jtile[:], enc_p[:, j, :], act.Tanh,
                                    bias=pred_p[:, j, u:u + 1], scale=1.0,
                                )
                            else:
                                nc.gpsimd.memset(jtile[:, :1], 0.0)
                            jt.append(jtile)
                        # j-outer, h-inner: each joint tile jt[j] is consumed
                        # in one burst, so ScalarE only has to deliver one
                        # tile per ~0.9us of PE work (helps the early ramp).
                        pts = [ps2.tile([128, V], F32, tag="ps2", name=f"mm_{rep}_{u}_{h}")
                               for h in range(HT)]
                        if not _OPTS["no_mm2"]:
                            for j in range(KJ):
                                for h in range(HT):
                                    if _OPTS["wide_mm"]:
                                        nc.tensor.matmul(
                                            pts[h][:],
                                            jt[j][:, h * 128:(h + 1) * 128],
                                            wj_sb[:, j, :],
                                            start=(j == 0),
                                            stop=(j == KJ - 1),
                                        )
                                    else:
                                        for vh in range(2):
                                            nc.tensor.matmul(
                                                pts[h][:, vh * 512:(vh + 1) * 512],
                                                jt[j][:, h * 128:(h + 1) * 128],
                                                wj_sb[:, j, vh * 512:(vh + 1) * 512],
                                                start=(j == 0),
                                                stop=(j == KJ - 1),
                                            )
                        else:
                            for h in range(HT):
                                for vh in range(2):
                                    nc.tensor.matmul(
                                        pts[h][:, vh * 512:(vh + 1) * 512],
                                        jt[0][:, h * 128:(h + 1) * 128],
                                        wj_sb[:, 0, vh * 512:(vh + 1) * 512],
                                        start=True, stop=True,
                                    )
                        for h in range(HT):
                            if not _OPTS["no_drain"]:
                                nc.vector.tensor_tensor(
                                    ot[:, h, ui * V:(ui + 1) * V],
                                    pts[h][:],
                                    bj_sb[:],
                                    mybir.AluOpType.add,
                                )
                            else:
                                nc.vector.tensor_copy(ot[:, h, ui * V:ui * V + 1], pts[h][:, :1])
                    if not _OPTS["no_store"]:
                        nc.sync.dma_start(
                            out_ap[:, u_base:u_base + gsz, :].rearrange(
                                "(h p) u v -> p h (u v)", p=128),
                            ot[:],
                        )
                    u_base += gsz


def _build_program(repeats=1):
    nc = bacc.Bacc("TRN2", target_bir_lowering=False, debug=False, num_devices=N_CORES)
    d = {
        "enc_tT": nc.dram_tensor("enc_tT", (DE, T), BF16, kind="ExternalInput"),
        "pred_tT": nc.dram_tensor("pred_tT", (DP, U), BF16, kind="ExternalInput"),
        "W_enc": nc.dram_tensor("W_enc", (DE, J), BF16, kind="ExternalInput"),
        "W_pred": nc.dram_tensor("W_pred", (DP, J), BF16, kind="ExternalInput"),
        "W_joint": nc.dram_tensor("W_joint", (J, V), BF16, kind="ExternalInput"),
        "b_sum": nc.dram_tensor("b_sum", (J,), F32, kind="ExternalInput"),
        "b_joint": nc.dram_tensor("b_joint", (V,), F32, kind="ExternalInput"),
        "logits": nc.dram_tensor("logits", (T, U, V), BF16, kind="ExternalOutput"),
    }
    with tile.TileContext(nc) as tc:
        _emit(tc, nc, d, repeats=repeats)
    nc.compile()
    return nc


def _prep_in_maps(enc_out, pred_out, W_enc, b_enc, W_pred, b_pred, W_joint, b_joint):
    bf = mybir.dt.np(BF16)
    shared = {
        "W_enc": np.ascontiguousarray(np.asarray(W_enc, dtype=np.float32).astype(bf)),
        "W_pred": np.ascontiguousarray(np.asarray(W_pred, dtype=np.float32).astype(bf)),
        "W_joint": np.ascontiguousarray(np.asarray(W_joint, dtype=np.float32).astype(bf)),
        "b_sum": np.asarray(b_enc, dtype=np.float32) + np.asarray(b_pred, dtype=np.float32),
        "b_joint": np.ascontiguousarray(b_joint, dtype=np.float32),
    }
    return [
        {
            "enc_tT": np.ascontiguousarray(
                np.asarray(enc_out[c], dtype=np.float32).astype(bf).T),
            "pred_tT": np.ascontiguousarray(
                np.asarray(pred_out[c], dtype=np.float32).astype(bf).T),
            **shared,
        }
        for c in range(N_CORES)
    ]


def kernel(enc_out, pred_out, W_enc, b_enc, W_pred, b_pred, W_joint, b_joint):
    nc = _CACHE.get("nc")
    if nc is None:
        nc = _CACHE["nc"] = _build_program()
    in_maps = _prep_in_maps(enc_out, pred_out, W_enc, b_enc, W_pred, b_pred,
                            W_joint, b_joint)
    res = bass_utils.run_bass_kernel_spmd(nc, in_maps, core_ids=list(range(N_CORES)))
    _CACHE["last_results"] = res
    return np.stack([np.asarray(res.results[c]["logits"]).astype(np.float32)
                     for c in range(N_CORES)])


# revision 28
# speedup vs baseline: 1.1891x; 1.0052x over previous
"""Trainium2 Bass kernel for an RNN-T joint network.

Computation (per batch element b):
    enc_p  = enc_out @ W_enc + b_enc                      # (T, J)
    pred_p = pred_out @ W_pred + b_pred                   # (U, J)
    joint  = tanh(enc_p[:, None, :] + pred_p[None, :, :]) # (T, U, J)
    logits = joint @ W_joint + b_joint                    # (T, U, V)

Distribution: data-parallel over the batch dim B=8, one batch element per
NeuronCore.  J-major layout (J on the SBUF partition dim) so the broadcast
add + tanh fuses into one ScalarE activation per (j-tile, u) and the vocab
matmul contracts over the partition dim.

Key optimizations (steady state measures ~225us/iter vs the ~218us bf16
tensor-engine roofline for the 17.2 GFLOP/core vocab projection):
  * inputs are pre-transposed and pre-cast to bf16 on the host (enc/pred
    arrive as (D, T)/(D, U); weights as bf16) - no PE transposes, no
    on-device casts, half the input DMA bytes.  b_enc+b_pred summed on host.
  * logits stored as bf16 (halves the 64 MiB/core store traffic; host
    upcasts to fp32).  bf16 math keeps L2 rel err ~4e-3, far under the
    2e-2 gate.
  * stores batched 4 u's per DMA -> stores of 2 MiB with 8 KiB contiguous
    DRAM chunks; the last group is split so the unhidden final store is
    small.
  * PSUM as [128, 1024] two-bank tiles; one VectorE drain (bias add,
    fp32->bf16) per (u, t-half).
  * inputs packed on the host into partition-major blobs laid out exactly
    as SBUF wants them, so each load DMA moves one large contiguous chunk
    per partition (128 descriptors/DMA; DMA issue costs ~8ns/descriptor,
    so fine-grained loads serialized ~20us of issue on the Pool engine).
    Loads ride two queues; W_joint is split per j-tile so the first vocab
    matmul starts as soon as wj[0] lands.
  * each joint tile is consumed in one 4-matmul burst so ScalarE tanh
    production stays ahead of PE demand.
"""

from contextlib import ExitStack

import numpy as np

import concourse.bacc as bacc
import concourse.mybir as mybir
import concourse.tile as tile
import concourse.bass_utils as bass_utils

N_CORES = 8
T, U, J, V = 256, 64, 512, 1024
DE, DP = 512, 640
KJ = J // 128    # j-tiles
KE = DE // 128   # enc contraction tiles
KP = DP // 128   # pred contraction tiles
HT = T // 128    # t-halves
F32 = mybir.dt.float32
BF16 = mybir.dt.bfloat16

# Offsets (in elements, per partition) into the packed bf16 input blob.
# The host lays inputs out partition-major so every load DMA moves one
# large contiguous chunk per partition (128 descriptors per DMA instead
# of thousands of 512B-2KB ones - DMA issue costs ~8ns/descriptor).
OFF_ENC = 0                      # [KE, T]   enc_out^T
OFF_WENC = OFF_ENC + KE * T      # [KE, J]   W_enc
OFF_PRED = OFF_WENC + KE * J     # [KP, U]   pred_out^T
OFF_WPRED = OFF_PRED + KP * U    # [KP, J]   W_pred
OFF_WJ = OFF_WPRED + KP * J      # [KJ, V]   W_joint
NB16 = OFF_WJ + KJ * V
NB32 = KJ + V                    # [KJ] b_enc+b_pred | [V] b_joint (replicated)

_CACHE: dict = {}
# Ablation switches for bench/devloop only.
_OPTS: dict = {"no_store": False, "no_mm2": False, "no_act": False, "no_drain": False,
               "wide_mm": False,  # N=1024 matmul fails the s3d3 ISA check (PSUM bank)
               "prewarm": 0,
               "group": 4, "ps_bufs": 4, "joint_bufs": 12, "out_bufs": 2}


def _emit(tc, nc, d, repeats=1):
    act = mybir.ActivationFunctionType
    g = _OPTS["group"]
    ng = U // g
    with ExitStack() as ctx:
        const = ctx.enter_context(tc.tile_pool(name="const", bufs=1))
        stg = ctx.enter_context(tc.tile_pool(name="stg", bufs=1 if repeats == 1 else 2))
        joint_pool = ctx.enter_context(tc.tile_pool(name="jp", bufs=_OPTS["joint_bufs"]))
        out_pool = ctx.enter_context(tc.tile_pool(name="op", bufs=_OPTS["out_bufs"]))

        # ---------------- loads: packed blobs, two queues ----------------
        # gpsimd queue: stage-1 inputs + biases; sync queue: pred inputs +
        # W_joint (split per j so the first mm2 starts when wj[0] lands).
        mega16 = const.tile([128, NB16], BF16, tag="mega16")
        mega32 = const.tile([128, NB32], F32, tag="mega32")
        b16 = d["blob16"].ap()
        nc.gpsimd.dma_start(mega16[:, OFF_ENC:OFF_PRED], b16[:, OFF_ENC:OFF_PRED])
        nc.gpsimd.dma_start(mega32[:], d["blob32"].ap())
        nc.sync.dma_start(mega16[:, OFF_PRED:OFF_WJ], b16[:, OFF_PRED:OFF_WJ])
        for j in range(KJ):
            o = OFF_WJ + j * V
            nc.sync.dma_start(mega16[:, o:o + V], b16[:, o:o + V])

        def enc_t(k):
            return mega16[:, OFF_ENC + k * T:OFF_ENC + (k + 1) * T]

        def wenc(k, j):
            o = OFF_WENC + k * J + j * 128
            return mega16[:, o:o + 128]

        def pred_t(k):
            return mega16[:, OFF_PRED + k * U:OFF_PRED + (k + 1) * U]

        def wpred(k, j):
            o = OFF_WPRED + k * J + j * 128
            return mega16[:, o:o + 128]

        def wj(j, v0, v1):
            o = OFF_WJ + j * V
            return mega16[:, o + v0:o + v1]

        bj_sb = mega32[:, KJ:KJ + V]

        if _OPTS["prewarm"]:
            # Keep PE busy during the input-DMA phase so the HAM clock gate
            # un-throttles (K=8/8) before the real matmul stream begins.
            junk = const.tile([128, 640], BF16, tag="pw_junk")
            nc.gpsimd.memset(junk[:], 0.0)
            with tc.tile_pool(name="pw_ps", bufs=1, space="PSUM") as pwp:
                pw = pwp.tile([128, 512], F32, tag="pw")
                for _ in range(_OPTS["prewarm"]):
                    nc.tensor.matmul(pw[:], junk[:, :128], junk[:, 128:640],
                                     start=True, stop=True)

        out_ap = d["logits"].ap()
        for rep in range(repeats):
            with tc.tile_pool(name=f"ps1_{rep}", bufs=2, space="PSUM") as ps1:
                # ---------------- stage-1 projections ----------------
                # j-outer: enc_p[0] / pred_p[0] complete early so the
                # ScalarE tanh pipeline starts filling immediately.
                # Interleave pred after the first enc j so ACT(u=0) can
                # begin as soon as (enc_p[0], pred_p[0]) exist.
                enc_p = stg.tile([128, KJ, T], F32, tag="enc_p", name=f"enc_p_{rep}")
                pred_p = stg.tile([128, KJ, U], F32, tag="pred_p", name=f"pred_p_{rep}")
                for j in range(KJ):
                    pt = ps1.tile([128, T], F32, tag="ps1mm", name=f"mm_enc_{rep}_{j}")
                    for k in range(KE):
                        nc.tensor.matmul(
                            pt[:],
                            wenc(k, j),
                            enc_t(k),
                            start=(k == 0),
                            stop=(k == KE - 1),
                        )
                    nc.vector.tensor_copy(enc_p[:, j, :], pt[:])
                    # pred_p[j] right after enc_p[j]; carries b_enc + b_pred.
                    pp = ps1.tile([128, U], F32, tag="ps1pu", name=f"mm_pred_{rep}_{j}")
                    for k in range(KP):
                        nc.tensor.matmul(
                            pp[:],
                            wpred(k, j),
                            pred_t(k),
                            start=(k == 0),
                            stop=(k == KP - 1),
                        )
                    nc.vector.tensor_scalar_add(pred_p[:, j, :], pp[:], mega32[:, j:j + 1])

            # ---------------- main loop over u, grouped stores ----------
            # Tapered tail: the last group is split 2+1+1 and the final
            # stores alternate DMA queues, so the unhidden tail after the
            # last matmul is one small parallel store pair.
            groups = [g] * (ng - 1) + [2, 1, 1] if g == 4 else [g] * ng
            assert sum(groups) == U
            with tc.tile_pool(name=f"ps2_{rep}", bufs=_OPTS["ps_bufs"], space="PSUM") as ps2:
                u_base = 0
                for gi, gsz in enumerate(groups):
                    ot = out_pool.tile([128, HT, gsz * V], BF16,
                                       tag=f"out{gsz}", name=f"out_{rep}_{gi}")
                    for ui in range(gsz):
                        u = u_base + ui
                        jt = []
                        for j in range(KJ):
                            jtile = joint_pool.tile([128, T], BF16, tag="joint",
                                                    name=f"joint_{rep}_{u}_{j}")
                            if not _OPTS["no_act"]:
                                nc.scalar.activation(
                                    jtile[:], enc_p[:, j, :], act.Tanh,
                                    bias=pred_p[:, j, u:u + 1], scale=1.0,
                                )
                            else:
                                nc.gpsimd.memset(jtile[:, :1], 0.0)
                            jt.append(jtile)
                        # j-outer, h-inner: each joint tile jt[j] is consumed
                        # in one burst, so ScalarE only has to deliver one
                        # tile per ~0.9us of PE work (helps the early ramp).
                        pts = [ps2.tile([128, V], F32, tag="ps2", name=f"mm_{rep}_{u}_{h}")
                               for h in range(HT)]
                        if not _OPTS["no_mm2"]:
                            for j in range(KJ):
                                for h in range(HT):
                                    for vh in range(2):
                                        nc.tensor.matmul(
                                            pts[h][:, vh * 512:(vh + 1) * 512],
                                            jt[j][:, h * 128:(h + 1) * 128],
                                            wj(j, vh * 512, (vh + 1) * 512),
                                            start=(j == 0),
                                            stop=(j == KJ - 1),
                                        )
                        else:
                            for h in range(HT):
                                for vh in range(2):
                                    nc.tensor.matmul(
                                        pts[h][:, vh * 512:(vh + 1) * 512],
                                        jt[0][:, h * 128:(h + 1) * 128],
                                        wj(0, vh * 512, (vh + 1) * 512),
                                        start=True, stop=True,
                                    )
                        for h in range(HT):
                            if not _OPTS["no_drain"]:
                                nc.vector.tensor_tensor(
                                    ot[:, h, ui * V:(ui + 1) * V],
                                    pts[h][:],
                                    bj_sb,
                                    mybir.AluOpType.add,
                                )
                            else:
                                nc.vector.tensor_copy(ot[:, h, ui * V:ui * V + 1], pts[h][:, :1])
                    if not _OPTS["no_store"]:
                        # Last store rides the (idle) gpsimd queue so the
                        # final two stores drain in parallel.
                        eng = nc.gpsimd if gi == len(groups) - 1 else nc.sync
                        eng.dma_start(
                            out_ap[:, u_base:u_base + gsz, :].rearrange(
                                "(h p) u v -> p h (u v)", p=128),
                            ot[:],
                        )
                    u_base += gsz


def _build_program(repeats=1):
    nc = bacc.Bacc("TRN2", target_bir_lowering=False, debug=False, num_devices=N_CORES)
    d = {
        "blob16": nc.dram_tensor("blob16", (128, NB16), BF16, kind="ExternalInput"),
        "blob32": nc.dram_tensor("blob32", (128, NB32), F32, kind="ExternalInput"),
        "logits": nc.dram_tensor("logits", (T, U, V), BF16, kind="ExternalOutput"),
    }
    with tile.TileContext(nc) as tc:
        _emit(tc, nc, d, repeats=repeats)
    nc.compile()
    return nc


def _pack16(arr, k):
    """(k*128, N) -> (128, k*N): partition p holds rows p, 128+p, ..."""
    n = arr.shape[1]
    return arr.reshape(k, 128, n).transpose(1, 0, 2).reshape(128, k * n)


def _prep_in_maps(enc_out, pred_out, W_enc, b_enc, W_pred, b_pred, W_joint, b_joint):
    bf = mybir.dt.np(BF16)
    wenc_p = _pack16(np.asarray(W_enc, dtype=np.float32).astype(bf), KE)
    wpred_p = _pack16(np.asarray(W_pred, dtype=np.float32).astype(bf), KP)
    wj_p = _pack16(np.asarray(W_joint, dtype=np.float32).astype(bf), KJ)
    bsum = (np.asarray(b_enc, dtype=np.float32)
            + np.asarray(b_pred, dtype=np.float32)).reshape(KJ, 128).T
    bj = np.broadcast_to(np.asarray(b_joint, dtype=np.float32), (128, V))
    blob32 = np.ascontiguousarray(
        np.concatenate([bsum, bj], axis=1), dtype=np.float32)
    in_maps = []
    for c in range(N_CORES):
        enc_p = _pack16(np.asarray(enc_out[c], dtype=np.float32).astype(bf).T, KE)
        pred_p = _pack16(np.asarray(pred_out[c], dtype=np.float32).astype(bf).T, KP)
        blob16 = np.ascontiguousarray(
            np.concatenate([enc_p, wenc_p, pred_p, wpred_p, wj_p], axis=1))
        in_maps.append({"blob16": blob16, "blob32": blob32})
    return in_maps


def kernel(enc_out, pred_out, W_enc, b_enc, W_pred, b_pred, W_joint, b_joint):
    nc = _CACHE.get("nc")
    if nc is None:
        nc = _CACHE["nc"] = _build_program()
    in_maps = _prep_in_maps(enc_out, pred_out, W_enc, b_enc, W_pred, b_pred,
                            W_joint, b_joint)
    res = bass_utils.run_bass_kernel_spmd(nc, in_maps, core_ids=list(range(N_CORES)))
    _CACHE["last_results"] = res
    return np.stack([np.asarray(res.results[c]["logits"]).astype(np.float32)
                     for c in range(N_CORES)])
